# revision 3
# baseline (speedup 1.0000x reference)
"""Trainium2 Bass kernel for jagged positional-encoding gather+add.

out[b, t] = x[b, t] + pe[pos[b, t]]  for t < lengths[b], else 0.

Sharding: data-parallel over batch B=32 across 8 NeuronCores (4 batches
per core); the PE table is replicated. Per core, for each batch:
  - DMA x[b] into SBUF as [128 partitions, 32, 256] (token t = p*32 + n)
  - gpsimd.dma_gather pulls the 4096 indexed PE rows from HBM into the
    matching layout (gather slot i lands on partition i%128, col i//128,
    so the host pre-permutes indices: slot[c*128+p] = pos[b, p*32+c])
  - DVE adds x + pe_rows; a per-token {0,1} mask column (built from an
    iota vs. lengths compare) zeroes the padding via per-partition
    tensor_scalar/activation multiplies
  - DMA the result back out.
"""

import sys

for _p in ("/opt/trn_rl_repo",):
    if _p not in sys.path:
        sys.path.append(_p)

import numpy as np

B = 32
L = 4096
D = 256
MAX_LEN = 5000
N_CORES = 8
BPC = B // N_CORES          # batches per core
NT = L // 128               # tokens per partition (free-dim groups)

_CACHE = {}


def _build_nc():
    import concourse.bacc as bacc
    import concourse.mybir as mybir
    import concourse.tile as tile

    nc = bacc.Bacc("TRN2", target_bir_lowering=False, debug=False,
                   num_devices=N_CORES)
    f32 = mybir.dt.float32
    xs = nc.dram_tensor("xs", [BPC, L, D], f32, kind="ExternalInput")
    pe = nc.dram_tensor("pe", [MAX_LEN, D], f32, kind="ExternalInput")
    idx = nc.dram_tensor("idx", [BPC, 128, L // 16], mybir.dt.int16,
                         kind="ExternalInput")
    lens = nc.dram_tensor("lens", [128, BPC], f32, kind="ExternalInput")
    out = nc.dram_tensor("out", [BPC, L, D], f32, kind="ExternalOutput")

    xs_ap, pe_ap, idx_ap, lens_ap, out_ap = (t.ap() for t in (xs, pe, idx, lens, out))

    with tile.TileContext(nc) as tc:
        with (
            tc.tile_pool(name="cpool", bufs=1) as cpool,
            tc.tile_pool(name="dpool", bufs=2) as dpool,
            tc.tile_pool(name="spool", bufs=2) as spool,
        ):
            lens_sb = cpool.tile([128, BPC], f32)
            nc.sync.dma_start(lens_sb[:, :], lens_ap[:, :])
            iota_i = cpool.tile([128, NT], mybir.dt.int32)
            nc.gpsimd.iota(iota_i[:, :], pattern=[[1, NT]], base=0,
                           channel_multiplier=NT)
            iota_f = cpool.tile([128, NT], f32)
            nc.vector.tensor_copy(iota_f[:, :], iota_i[:, :])

            for b in range(BPC):
                x_t = dpool.tile([128, NT, D], f32, tag="x")
                pe_t = dpool.tile([128, NT, D], f32, tag="pe")
                idx_t = spool.tile([128, L // 16], mybir.dt.int16, tag="idx")
                mask_t = spool.tile([128, NT], f32, tag="mask")

                nc.sync.dma_start(
                    x_t[:, :, :],
                    xs_ap[b].rearrange("(p n) d -> p n d", p=128),
                )
                nc.sync.dma_start(idx_t[:, :], idx_ap[b])
                # SWDGE descriptor-ring capacity caps one gather call at
                # ~1024 indices (65 descs/DMA); 4096 in one call wedges the
                # device. Split into 4 calls of 1024.
                GCHUNK = 1024
                for k in range(L // GCHUNK):
                    ng = GCHUNK // 128          # 8 free-dim groups per call
                    nc.gpsimd.dma_gather(
                        pe_t[:, k * ng:(k + 1) * ng, :], pe_ap[:, :],
                        idx_t[:, k * (GCHUNK // 16):(k + 1) * (GCHUNK // 16)],
                        num_idxs=GCHUNK, num_idxs_reg=GCHUNK, elem_size=D,
                    )
                # mask[p, n] = 1.0 if p*NT + n < len(b) else 0.0
                nc.vector.tensor_scalar(
                    mask_t[:, :], iota_f[:, :], lens_sb[:, b:b + 1], None,
                    op0=mybir.AluOpType.is_lt,
                )
                nc.vector.tensor_tensor(
                    x_t[:, :, :], x_t[:, :, :], pe_t[:, :, :],
                    op=mybir.AluOpType.add,
                )
                for n in range(NT):
                    if n % 2 == 0:
                        nc.vector.tensor_scalar_mul(
                            x_t[:, n, :], x_t[:, n, :], mask_t[:, n:n + 1])
                    else:
                        nc.scalar.mul(x_t[:, n, :], x_t[:, n, :],
                                      mask_t[:, n:n + 1])
                nc.sync.dma_start(
                    out_ap[b].rearrange("(p n) d -> p n d", p=128),
                    x_t[:, :, :],
                )
    nc.compile()
    return nc


def _get_nc():
    if "nc" not in _CACHE:
        _CACHE["nc"] = _build_nc()
    return _CACHE["nc"]


def kernel(x, pe, pos, lengths):
    from concourse.bass_utils import run_bass_kernel_spmd

    x = np.asarray(x, dtype=np.float32)
    pe = np.ascontiguousarray(np.asarray(pe, dtype=np.float32))
    pos16 = np.asarray(pos).astype(np.int16)          # values < 4096 fit
    lens_f = np.asarray(lengths).astype(np.float32)

    nc = _get_nc()
    in_maps = []
    for c in range(N_CORES):
        bs = slice(c * BPC, (c + 1) * BPC)
        # gather slot i = c*128 + p must hold pos[b, p*NT + c]
        slot = pos16[bs].reshape(BPC, 128, NT).transpose(0, 2, 1).reshape(BPC, L)
        wrapped = slot.reshape(BPC, L // 16, 16).transpose(0, 2, 1)  # [BPC,16,L/16]
        idx = np.ascontiguousarray(
            np.tile(wrapped, (1, 8, 1)))                  # [BPC,128,L/16]
        lens_rep = np.ascontiguousarray(
            np.broadcast_to(lens_f[bs][None, :], (128, BPC)))
        in_maps.append({
            "xs": np.ascontiguousarray(x[bs]),
            "pe": pe,
            "idx": idx,
            "lens": lens_rep,
        })

    res = run_bass_kernel_spmd(nc, in_maps, core_ids=list(range(N_CORES)))
    return np.concatenate([res.results[c]["out"] for c in range(N_CORES)], axis=0)
